# revision 27
# baseline (speedup 1.0000x reference)
"""CrossLayer (DCN-v2 style) Trainium2 kernel.

Computes  out = x0 * (xl . W)[:, None] + b + xl   for x0, xl [16384, 4096],
W, b [4096] — data-parallel over 8 NeuronCores (2048 rows each, W/b
replicated). The 2e-2 rel-err gate leaves ~4 decades of precision headroom,
so the whole data path runs in fp16 (worst-case abs err ~0.7 vs ~11.8
allowed): HBM traffic halves from 96MB to 48MB per core, and fp16 also
unlocks the DVE 2x_1P perf mode.

Per-core dataflow, per 128-row full-width tile (rows on partitions, d free):
  DVE  tensor_mul    t  = xl * W_bcast        (2x fp16 mode, ~2.3us)
  ACT  activation    s  = accum_out(copy t)   (free-axis sum on ScalarE,
                                               dead copy lands in PSUM)
  DVE  tensor_add    u  = xl + b_bcast        (2x)
  ACT  activation    v  = x0 * s              (per-partition scale, ~3.7us;
                                               3 of 16 tiles use DVE
                                               tensor_scalar at 4x, 1.28us)
  DVE  tensor_add    o  = v + u               (2x, emitted 2 tiles behind)
DVE runs ONLY ops from its 2x/4x-capable set {tensor_tensor,
tensor_scalar, copy/cast}: scalar_tensor_tensor (the baseline's workhorse)
and tensor_reduce are NOT in that set and run at 1x, which made both the
fused 3-pass version and a DVE-reduce version DVE-bound at ~207us. The
row-sum and most x0*s scales ride ScalarE; the split puts ~126us on each
engine. Measured 142-173us wall depending on environment congestion.

Pipeline discipline (engines execute their streams IN ORDER — all
measured, each worth 10-25us):
  - o(i)/store(i) are emitted LAG=2 tiles behind the producer front, so
    DVE reaches o(i) only after ScalarE has slack to finish v(i);
  - x0_s gets 6 buffers: its slot is recycled by v(i), the LAST consumer
    on the slowest engine, and with shallow buffers the x0 load for tile
    i+3 head-of-line-blocks the in-order load ring, starving mul;
  - stores ride the GpSimd SWDGE ring: an issuing compute engine's
    sequencer would stall on o(i) readiness each tile.

Full-width tiles: a [128, 4096] fp16 tile is ONE contiguous 1MB block in
HBM (tile rows are consecutive full matrix rows), so DMA engines get
large contiguous 8KB packets instead of 4KB strided ones (292GB/s vs
205GB/s measured on the load ring).

W/b are replicated across partitions by stride-0 broadcast DMA on the ACT
HWDGE queue at t=0 (the PE ones-outer-product preamble it replaces cost
11us of drain passes on DVE/ScalarE plus ~5us of startup latency).
"""

import numpy as np

import concourse.bass as bass
import concourse.mybir as mybir
from concourse.bass_utils import run_bass_kernel_spmd
from concourse.tile import TileContext

N_CORES = 8
B, D = 16384, 4096
ROWS = B // N_CORES  # rows per core
P = 128
N_TILES = ROWS // P  # 16
FP32 = mybir.dt.float32
FP16 = mybir.dt.float16

_PROGRAM = None
LAST_RESULT = None  # test harness reads .exec_time_ns off this


def _split_multi_waits(nc: bass.Bass) -> None:
    """The staged neuronxcc walrus encodes at most ONE sync-wait per
    instruction ("Too many sync wait commands"); Tile's scheduler emits
    instructions waiting on several semaphores. Hoist the extra waits onto
    same-engine NoOps inserted immediately before — the sequencer blocks on
    each in turn, which is semantically identical."""
    n = 0
    for fn in nc.m.functions:
        for blk in fn.blocks:
            new_insts = []
            for inst in blk.instructions:
                si = inst.sync_info
                waits = list(si.on_wait) if si is not None and si.on_wait else []
                if len(waits) > 1:
                    for w in waits[:-1]:
                        nop = mybir.InstNoOp(
                            name=f"{inst.name}-waitsplit-{n}",
                            engine=inst.engine,
                            ins=[],
                            outs=[],
                            sync_info=mybir.SyncInfo(on_wait=[w], on_update=[]),
                        )
                        new_insts.append(nop)
                        n += 1
                    inst.sync_info = mybir.SyncInfo(
                        on_wait=[waits[-1]], on_update=list(si.on_update or [])
                    )
                new_insts.append(inst)
            blk.instructions = new_insts


def _build_program() -> bass.Bass:
    nc = bass.Bass()
    x0 = nc.declare_dram_parameter("x0", [ROWS, D], FP16, isOutput=False)
    xl = nc.declare_dram_parameter("xl", [ROWS, D], FP16, isOutput=False)
    W = nc.declare_dram_parameter("W", [D], FP16, isOutput=False)
    b = nc.declare_dram_parameter("b", [D], FP16, isOutput=False)
    out = nc.declare_dram_parameter("out", [ROWS, D], FP16, isOutput=True)

    x0_t = x0[:, :].rearrange("(n p) d -> n p d", p=P)
    xl_t = xl[:, :].rearrange("(n p) d -> n p d", p=P)
    out_t = out[:, :].rearrange("(n p) d -> n p d", p=P)
    w_row = W[:].rearrange("(r d) -> r d", r=1)
    b_row = b[:].rearrange("(r d) -> r d", r=1)

    MUL = mybir.AluOpType.mult
    ADD = mybir.AluOpType.add
    COPY = mybir.ActivationFunctionType.Copy

    with TileContext(nc) as tc:
        with (
            tc.tile_pool(name="consts", bufs=1) as cpool,
            tc.tile_pool(name="io", bufs=3) as iopool,
            tc.tile_pool(name="work", bufs=2) as wpool,
            tc.tile_pool(name="psum", bufs=1, space="PSUM") as ppool,
        ):
            w_b = cpool.tile([P, D], FP16)
            b_b = cpool.tile([P, D], FP16)
            # Replicate W/b across partitions with stride-0 broadcast DMA
            # (128 re-reads of one fp16 8KB row = 1MB each, ~3us on the
            # store ring which is idle at t=0). Replaces the PE
            # ones-outer-product + PSUM-drain preamble, which cost 5.5us of
            # DVE CASTs, 5.5us of ScalarE drains, and ~12us of startup
            # latency before the first full-width mul could see w_b.
            # Issued from the ACT HWDGE queue at t=0 (ScalarE has nothing
            # else yet and the issues carry no waits) so the load ring's
            # first packets are xl(0)/x0(0), not 2MB of broadcast.
            nc.scalar.dma_start(
                out=w_b[:, :], in_=w_row.partition_broadcast(P)
            )
            nc.scalar.dma_start(
                out=b_b[:, :], in_=b_row.partition_broadcast(P)
            )

            # Engines execute their instruction streams IN ORDER, so if
            # o(i) — which waits on ScalarE's v(i) — sits right before
            # mul(i+1) in DVE's stream, a late v head-of-line-blocks the
            # whole DVE pipeline (measured ~40us of bubbles). Emit o/store
            # LAG tiles behind the producer front so DVE only reaches o(i)
            # after ScalarE has had LAG tiles of slack to finish v(i).
            LAG = 2
            pend = {}

            def emit_tail(i):
                x0h, uh, vh = pend.pop(i)
                o = wpool.tile([P, D], FP16, name="o", bufs=3)
                nc.vector.tensor_add(o[:, :], vh[:, :], uh[:, :])
                # Stores ride the GpSimd SWDGE ring: the issuing engine's
                # sequencer stalls until o(i) is ready, which on a compute
                # engine would serialize its pipeline behind DVE each tile
                # (measured +25us when stores were issued by ScalarE).
                nc.gpsimd.dma_start(out=out_t[i], in_=o[:, :])

            for i in range(N_TILES):
                xl_s = iopool.tile([P, D], FP16, name="xl_s", bufs=3)
                x0_s = iopool.tile([P, D], FP16, name="x0_s", bufs=6)
                nc.sync.dma_start(out=xl_s[:, :], in_=xl_t[i])
                nc.sync.dma_start(out=x0_s[:, :], in_=x0_t[i])

                t1 = wpool.tile([P, D], FP16, name="t1", bufs=3)
                nc.vector.tensor_mul(t1[:, :], xl_s[:, :], w_b[:, :])
                # Row-dot: ScalarE's free-axis accumulator sums t1 while
                # copying it to a junk tile (the copy output is dead).
                junk = ppool.tile([P, D], FP32, name="junk", tag="junk")
                s = wpool.tile([P, 1], FP32, name="s", bufs=8)
                nc.scalar.activation(
                    junk[:, :], t1[:, :], COPY, bias=0.0, accum_out=s[:, :]
                )
                u = wpool.tile([P, D], FP16, name="u", bufs=4)
                nc.vector.tensor_add(u[:, :], xl_s[:, :], b_b[:, :])
                v = wpool.tile([P, D], FP16, name="v", bufs=4)
                # ScalarE (1 elem/cycle/lane, ~3.7us/pass) saturates doing
                # reduce+scale for every tile; shift 3 of 16 scale passes to
                # DVE tensor_scalar (4x fp16 mode, measured 1.28us) to
                # balance the engines (~124us each incl. semaphore
                # overheads). NOTE: variants that lag the TS into the tail,
                # move tiles around (4,10,15), or split the w broadcast all
                # REGRESSED to ~166us — the pool-rotation rhythm matters
                # more than the ~5us these chased.
                # Tile 15 is a TS tile: in the closing chain there are no
                # later muls for its s-wait to block, and it drops ScalarE's
                # 3.7us v from the serial tail.
                if i in (4, 9, 15):
                    nc.vector.tensor_scalar(
                        v[:, :], x0_s[:, :], s[:, :], None, MUL
                    )
                else:
                    nc.scalar.activation(
                        v[:, :], x0_s[:, :], COPY, bias=0.0, scale=s[:, :]
                    )
                pend[i] = (x0_s, u, v)
                if i >= LAG:
                    emit_tail(i - LAG)
            for i in range(N_TILES - LAG, N_TILES):
                emit_tail(i)
    _split_multi_waits(nc)
    return nc


def kernel(x0, xl, W, b, _trace=False, **trace_kwargs):
    global _PROGRAM, LAST_RESULT
    if _PROGRAM is None:
        _PROGRAM = _build_program()

    x0 = np.ascontiguousarray(np.asarray(x0, dtype=np.float16))
    xl = np.ascontiguousarray(np.asarray(xl, dtype=np.float16))
    W = np.ascontiguousarray(np.asarray(W, dtype=np.float16))
    b = np.ascontiguousarray(np.asarray(b, dtype=np.float16))

    in_maps = [
        {
            "x0": x0[c * ROWS : (c + 1) * ROWS],
            "xl": xl[c * ROWS : (c + 1) * ROWS],
            "W": W,
            "b": b,
        }
        for c in range(N_CORES)
    ]
    res = run_bass_kernel_spmd(
        _PROGRAM, in_maps, list(range(N_CORES)), trace=_trace, **trace_kwargs
    )
    LAST_RESULT = res
    return np.concatenate([r["out"] for r in res.results], axis=0).astype(np.float32)


# revision 28
# speedup vs baseline: 1.0701x; 1.0701x over previous
"""CrossLayer (DCN-v2 style) Trainium2 kernel.

Computes  out = x0 * (xl . W)[:, None] + b + xl   for x0, xl [16384, 4096],
W, b [4096] — data-parallel over 8 NeuronCores (2048 rows each, W/b
replicated). The 2e-2 rel-err gate leaves ~4 decades of precision headroom,
so the whole data path runs in fp16 (worst-case abs err ~0.7 vs ~11.8
allowed): HBM traffic halves from 96MB to 48MB per core, and fp16 also
unlocks the DVE 2x_1P perf mode.

Per-core dataflow, per 128-row full-width tile (rows on partitions, d free):
  DVE  tensor_mul    t  = xl * W_bcast        (2x fp16 mode, ~2.3us)
  ACT  activation    s  = accum_out(copy t)   (free-axis sum on ScalarE,
                                               dead copy lands in PSUM)
  DVE  tensor_add    u  = xl + b_bcast        (2x)
  ACT  activation    v  = x0 * s              (per-partition scale, ~3.7us;
                                               3 of 16 tiles use DVE
                                               tensor_scalar at 4x, 1.28us)
  DVE  tensor_add    o  = v + u               (2x, emitted 2 tiles behind)
DVE runs ONLY ops from its 2x/4x-capable set {tensor_tensor,
tensor_scalar, copy/cast}: scalar_tensor_tensor (the baseline's workhorse)
and tensor_reduce are NOT in that set and run at 1x, which made both the
fused 3-pass version and a DVE-reduce version DVE-bound at ~207us. The
row-sum and most x0*s scales ride ScalarE; the split puts ~126us on each
engine. Measured 142-173us wall depending on environment congestion.

Pipeline discipline (engines execute their streams IN ORDER — all
measured, each worth 10-25us):
  - o(i)/store(i) are emitted LAG=2 tiles behind the producer front, so
    DVE reaches o(i) only after ScalarE has slack to finish v(i);
  - x0_s gets 6 buffers: its slot is recycled by v(i), the LAST consumer
    on the slowest engine, and with shallow buffers the x0 load for tile
    i+3 head-of-line-blocks the in-order load ring, starving mul;
  - stores ride the GpSimd SWDGE ring: an issuing compute engine's
    sequencer would stall on o(i) readiness each tile.

Full-width tiles: a [128, 4096] fp16 tile is ONE contiguous 1MB block in
HBM (tile rows are consecutive full matrix rows), so DMA engines get
large contiguous 8KB packets instead of 4KB strided ones (292GB/s vs
205GB/s measured on the load ring).

W/b are replicated across partitions by stride-0 broadcast DMA on the ACT
HWDGE queue at t=0 (the PE ones-outer-product preamble it replaces cost
11us of drain passes on DVE/ScalarE plus ~5us of startup latency).
"""

import numpy as np

import concourse.bass as bass
import concourse.mybir as mybir
from concourse.bass_utils import run_bass_kernel_spmd
from concourse.tile import TileContext

N_CORES = 8
B, D = 16384, 4096
ROWS = B // N_CORES  # rows per core
P = 128
N_TILES = ROWS // P  # 16
FP32 = mybir.dt.float32
FP16 = mybir.dt.float16

_PROGRAM = None
LAST_RESULT = None  # test harness reads .exec_time_ns off this


def _split_multi_waits(nc: bass.Bass) -> None:
    """The staged neuronxcc walrus encodes at most ONE sync-wait per
    instruction ("Too many sync wait commands"); Tile's scheduler emits
    instructions waiting on several semaphores. Hoist the extra waits onto
    same-engine NoOps inserted immediately before — the sequencer blocks on
    each in turn, which is semantically identical."""
    n = 0
    for fn in nc.m.functions:
        for blk in fn.blocks:
            new_insts = []
            for inst in blk.instructions:
                si = inst.sync_info
                waits = list(si.on_wait) if si is not None and si.on_wait else []
                if len(waits) > 1:
                    for w in waits[:-1]:
                        nop = mybir.InstNoOp(
                            name=f"{inst.name}-waitsplit-{n}",
                            engine=inst.engine,
                            ins=[],
                            outs=[],
                            sync_info=mybir.SyncInfo(on_wait=[w], on_update=[]),
                        )
                        new_insts.append(nop)
                        n += 1
                    inst.sync_info = mybir.SyncInfo(
                        on_wait=[waits[-1]], on_update=list(si.on_update or [])
                    )
                new_insts.append(inst)
            blk.instructions = new_insts


def _build_program() -> bass.Bass:
    nc = bass.Bass()
    x0 = nc.declare_dram_parameter("x0", [ROWS, D], FP16, isOutput=False)
    xl = nc.declare_dram_parameter("xl", [ROWS, D], FP16, isOutput=False)
    W = nc.declare_dram_parameter("W", [D], FP16, isOutput=False)
    b = nc.declare_dram_parameter("b", [D], FP16, isOutput=False)
    out = nc.declare_dram_parameter("out", [ROWS, D], FP16, isOutput=True)

    x0_t = x0[:, :].rearrange("(n p) d -> n p d", p=P)
    xl_t = xl[:, :].rearrange("(n p) d -> n p d", p=P)
    out_t = out[:, :].rearrange("(n p) d -> n p d", p=P)
    w_row = W[:].rearrange("(r d) -> r d", r=1)
    b_row = b[:].rearrange("(r d) -> r d", r=1)

    MUL = mybir.AluOpType.mult
    ADD = mybir.AluOpType.add
    COPY = mybir.ActivationFunctionType.Copy

    with TileContext(nc) as tc:
        with (
            tc.tile_pool(name="consts", bufs=1) as cpool,
            tc.tile_pool(name="io", bufs=3) as iopool,
            tc.tile_pool(name="work", bufs=2) as wpool,
            tc.tile_pool(name="psum", bufs=1, space="PSUM") as ppool,
        ):
            w_b = cpool.tile([P, D], FP16)
            b_b = cpool.tile([P, D], FP16)
            # Replicate W/b across partitions with stride-0 broadcast DMA
            # (128 re-reads of one fp16 8KB row = 1MB each, ~3us on the
            # store ring which is idle at t=0). Replaces the PE
            # ones-outer-product + PSUM-drain preamble, which cost 5.5us of
            # DVE CASTs, 5.5us of ScalarE drains, and ~12us of startup
            # latency before the first full-width mul could see w_b.
            # Issued from the ACT HWDGE queue at t=0 (ScalarE has nothing
            # else yet and the issues carry no waits) so the load ring's
            # first packets are xl(0)/x0(0), not 2MB of broadcast.
            nc.scalar.dma_start(
                out=w_b[:, :], in_=w_row.partition_broadcast(P)
            )
            nc.scalar.dma_start(
                out=b_b[:, :], in_=b_row.partition_broadcast(P)
            )

            # Engines execute their instruction streams IN ORDER, so if
            # o(i) — which waits on ScalarE's v(i) — sits right before
            # mul(i+1) in DVE's stream, a late v head-of-line-blocks the
            # whole DVE pipeline (measured ~40us of bubbles). Emit o/store
            # LAG tiles behind the producer front so DVE only reaches o(i)
            # after ScalarE has had LAG tiles of slack to finish v(i).
            LAG = 2
            pend = {}

            def emit_tail(i):
                x0h, uh, vh = pend.pop(i)
                o = wpool.tile([P, D], FP16, name="o", bufs=3)
                nc.vector.tensor_add(o[:, :], vh[:, :], uh[:, :])
                # Stores ride the GpSimd SWDGE ring: the issuing engine's
                # sequencer stalls until o(i) is ready, which on a compute
                # engine would serialize its pipeline behind DVE each tile
                # (measured +25us when stores were issued by ScalarE).
                nc.gpsimd.dma_start(out=out_t[i], in_=o[:, :])

            for i in range(N_TILES):
                xl_s = iopool.tile([P, D], FP16, name="xl_s", bufs=3)
                x0_s = iopool.tile([P, D], FP16, name="x0_s", bufs=6)
                nc.sync.dma_start(out=xl_s[:, :], in_=xl_t[i])
                nc.sync.dma_start(out=x0_s[:, :], in_=x0_t[i])

                t1 = wpool.tile([P, D], FP16, name="t1", bufs=3)
                nc.vector.tensor_mul(t1[:, :], xl_s[:, :], w_b[:, :])
                # Row-dot: ScalarE's free-axis accumulator sums t1 while
                # copying it to a junk tile (the copy output is dead).
                junk = ppool.tile([P, D], FP32, name="junk", tag="junk")
                s = wpool.tile([P, 1], FP32, name="s", bufs=8)
                nc.scalar.activation(
                    junk[:, :], t1[:, :], COPY, bias=0.0, accum_out=s[:, :]
                )
                u = wpool.tile([P, D], FP16, name="u", bufs=4)
                nc.vector.tensor_add(u[:, :], xl_s[:, :], b_b[:, :])
                v = wpool.tile([P, D], FP16, name="v", bufs=4)
                # ScalarE (1 elem/cycle/lane, ~3.7us/pass) saturates doing
                # reduce+scale for every tile; shift 3 of 16 scale passes to
                # DVE tensor_scalar (4x fp16 mode, measured 1.28us) to
                # balance the engines (~124us each incl. semaphore
                # overheads). NOTE: variants that lag the TS into the tail,
                # move tiles around (4,10,15), or split the w broadcast all
                # REGRESSED to ~166us — the pool-rotation rhythm matters
                # more than the ~5us these chased.
                # Tile 15 is a TS tile: in the closing chain there are no
                # later muls for its s-wait to block, and it drops ScalarE's
                # 3.7us v from the serial tail.
                if i in (4, 9, 12, 15):
                    nc.vector.tensor_scalar(
                        v[:, :], x0_s[:, :], s[:, :], None, MUL
                    )
                else:
                    nc.scalar.activation(
                        v[:, :], x0_s[:, :], COPY, bias=0.0, scale=s[:, :]
                    )
                pend[i] = (x0_s, u, v)
                if i >= LAG:
                    emit_tail(i - LAG)
            for i in range(N_TILES - LAG, N_TILES):
                emit_tail(i)
    _split_multi_waits(nc)
    return nc


def kernel(x0, xl, W, b, _trace=False, **trace_kwargs):
    global _PROGRAM, LAST_RESULT
    if _PROGRAM is None:
        _PROGRAM = _build_program()

    x0 = np.ascontiguousarray(np.asarray(x0, dtype=np.float16))
    xl = np.ascontiguousarray(np.asarray(xl, dtype=np.float16))
    W = np.ascontiguousarray(np.asarray(W, dtype=np.float16))
    b = np.ascontiguousarray(np.asarray(b, dtype=np.float16))

    in_maps = [
        {
            "x0": x0[c * ROWS : (c + 1) * ROWS],
            "xl": xl[c * ROWS : (c + 1) * ROWS],
            "W": W,
            "b": b,
        }
        for c in range(N_CORES)
    ]
    res = run_bass_kernel_spmd(
        _PROGRAM, in_maps, list(range(N_CORES)), trace=_trace, **trace_kwargs
    )
    LAST_RESULT = res
    return np.concatenate([r["out"] for r in res.results], axis=0).astype(np.float32)
